# revision 16
# baseline (speedup 1.0000x reference)
"""Trainium2 Bass kernel for a 2-layer LSTM encoder/decoder forecaster.

Model (per batch element):
  teacher-forced over S=168 steps:  enc -> LSTM0 -> LSTM1 (keep last out)
  autoregressive rollout for 23 more steps feeding decoder output back.

Sharding: data-parallel, batch 1024 -> 8 cores x 128. All weights are
replicated and SBUF-resident; zero inter-core communication.

Layout: everything is FEATURE-MAJOR. Gates are computed as 16 chunks of
[128 gate-rows x 128 batch] PSUM tiles, with the (transposed, chunked)
weights as the stationary matmul operand and h / x as the moving
operand. Because the cell elementwise output h = sig(o)*tanh(c) is then
produced directly in [feature, batch] layout, it is ALREADY the k-chunk
operand the next step's recurrent matmuls need - no PE transposes, no
PSUM->SBUF copies of state anywhere in the loop.

Everything is bf16 on the matmul path (full PE rate at any width, and a
validated ~3e-3 end-to-end error vs the 2e-2 budget; fp8 was measured at
4.3e-2 and rejected). The cell state c stays fp32.

Gate chunk order after host-side row permutation: (f, i, o, g) so that
GA = [f|i] (chunks 0..7) and GB = [o|g] (chunks 8..15), letting each
activation read one contiguous PSUM span.

The encoder is algebraically fused into layer 0 (M0 = Wih0 @ W_enc, bias
folded into an appended ones-row of the feature-major input). Layer 1's
bias enters via 16 K=1 matmuls against a resident ones vector. The
decoder is augmented with a column that regenerates the ones-row so the
AR feedback tile needs no fixup at all: the decoder's SBUF output IS the
next step's input operand.

PE order per steady-state step: [xa(t), whh0(t), bias1(t)] prefetched at
the end of step t-1, then whh1(t), wih1(t) - chosen so every matmul's
input is ready before PE reaches it, keeping PE (the bottleneck at ~28.7k
cycles/step) gapless and at full p-state.
"""

import sys
import threading

sys.path.insert(0, "/opt/trn_rl_repo")

import numpy as np
import ml_dtypes

PRED_LEN = 24
F, I, H = 64, 128, 512
B, S = 1024, 168
NCORES = 8
BL = B // NCORES          # batch per core = 128
G = 4 * H                 # gate width 2048
NCH = G // 128            # 16 gate chunks
KCH = H // 128            # 4 k-chunks of the hidden dim
KX = F + 1                # x operand rows incl. ones row = 65
FD = F + 2                # decoder rows: 64 outputs + ones + pad = 66

BF16NP = ml_dtypes.bfloat16

_cache = {}
_cache_lock = threading.Lock()


def _gate_perm():
    # pytorch gate order i,f,g,o -> reorder rows to (f,i,o,g): chunks
    # 0-3=f, 4-7=i (-> GA), 8-11=o, 12-15=g (-> GB).
    return np.concatenate([
        np.arange(H, 2 * H),        # f
        np.arange(0, H),            # i
        np.arange(3 * H, 4 * H),    # o
        np.arange(2 * H, 3 * H),    # g
    ])


def _build_program(n_tf=S, n_ar=PRED_LEN - 1):
    import concourse.bacc as bacc
    import concourse.tile as tile
    import concourse.mybir as mybir

    F32 = mybir.dt.float32
    BF16 = mybir.dt.bfloat16
    AF = mybir.ActivationFunctionType

    nc = bacc.Bacc("TRN2", target_bir_lowering=False, debug=False,
                   num_devices=NCORES)

    x_d = nc.dram_tensor("xT", [KX, n_tf, BL], BF16, kind="ExternalInput").ap()
    m0_d = nc.dram_tensor("m0t", [KX, G], BF16, kind="ExternalInput").ap()
    whh0_d = nc.dram_tensor("whh0t", [128, KCH, G], BF16, kind="ExternalInput").ap()
    wih1_d = nc.dram_tensor("wih1t", [128, KCH, G], BF16, kind="ExternalInput").ap()
    whh1_d = nc.dram_tensor("whh1t", [128, KCH, G], BF16, kind="ExternalInput").ap()
    b1_d = nc.dram_tensor("b1", [1, G], BF16, kind="ExternalInput").ap()
    b1bc_d = nc.dram_tensor("b1bc", [128, NCH, BL], BF16, kind="ExternalInput").ap()
    ones_d = nc.dram_tensor("ones", [1, BL], BF16, kind="ExternalInput").ap()
    wdec_d = nc.dram_tensor("wdect", [128, KCH, FD], BF16, kind="ExternalInput").ap()
    bdec_d = nc.dram_tensor("bdec", [FD, 1], F32, kind="ExternalInput").ap()
    y_d = nc.dram_tensor("y", [n_ar + 1, F, BL], BF16, kind="ExternalOutput").ap()

    from contextlib import ExitStack
    with tile.TileContext(nc) as tc, ExitStack() as ctx:
        wpool = ctx.enter_context(tc.tile_pool(name="w", bufs=1))
        spool = ctx.enter_context(tc.tile_pool(name="s", bufs=2))
        hpool = ctx.enter_context(tc.tile_pool(name="h", bufs=2))
        dpool = ctx.enter_context(tc.tile_pool(name="d", bufs=2))
        gpool = ctx.enter_context(tc.tile_pool(name="g", bufs=1, space="PSUM"))

        # ---- resident weights + input ----
        # split the x load so step 0 only waits on the first few columns
        x_sb = wpool.tile([KX, n_tf, BL], BF16)
        x_head = min(8, n_tf)
        nc.sync.dma_start(x_sb[:, 0:x_head, :], x_d[:, 0:x_head, :])
        if x_head < n_tf:
            nc.sync.dma_start(x_sb[:, x_head:, :], x_d[:, x_head:, :])
        m0_sb = wpool.tile([KX, G], BF16)
        nc.sync.dma_start(m0_sb[:], m0_d[:])
        whh0_sb = wpool.tile([128, KCH, G], BF16)
        nc.sync.dma_start(whh0_sb[:], whh0_d[:])
        wih1_sb = wpool.tile([128, KCH, G], BF16)
        nc.sync.dma_start(wih1_sb[:], wih1_d[:])
        whh1_sb = wpool.tile([128, KCH, G], BF16)
        nc.sync.dma_start(whh1_sb[:], whh1_d[:])
        b1_sb = wpool.tile([1, G], BF16)
        nc.sync.dma_start(b1_sb[:], b1_d[:])
        b1bc_sb = wpool.tile([128, NCH, BL], BF16)
        nc.sync.dma_start(b1bc_sb[:], b1bc_d[:])
        ones_sb = wpool.tile([1, BL], BF16)
        nc.sync.dma_start(ones_sb[:], ones_d[:])
        wdec_sb = wpool.tile([128, KCH, FD], BF16)
        nc.sync.dma_start(wdec_sb[:], wdec_d[:])
        bdec_sb = wpool.tile([FD, 1], F32)
        nc.sync.dma_start(bdec_sb[:], bdec_d[:])

        # ga holds chunks 0-11 = f|i|o (3 PSUM banks), gb holds 12-15 = g
        # (1 bank): one sigmoid + one tanh cover all gate activations.
        def halves(ga, gb, m):
            return ga[:, m, :] if m < 12 else gb[:, m - 12, :]

        # start=True lazily zeroes the WHOLE 2KB PSUM bank (4 of our 512B
        # chunk regions), so only the first write per bank may set it; the
        # sibling regions still see the bank's pending-zero and overwrite.
        def emit_xa(ga, gb, rhs, only):
            # input-side gate contribution; `only`=True closes the group
            # (t=0 has no recurrent term).
            for m in range(NCH):
                nc.tensor.matmul(halves(ga, gb, m),
                                 m0_sb[:, m * 128:(m + 1) * 128], rhs,
                                 start=(m % 4 == 0), stop=only,
                                 skip_group_check=True)

        def emit_bias1(ga, gb, only):
            for m in range(NCH):
                nc.tensor.matmul(halves(ga, gb, m),
                                 b1_sb[:, m * 128:(m + 1) * 128], ones_sb[:],
                                 start=(m % 4 == 0), stop=only,
                                 skip_group_check=True)

        def emit_rec(w_sb, h, ga, gb, last):
            for m in range(NCH):
                out = halves(ga, gb, m)
                for k in range(KCH):
                    nc.tensor.matmul(out,
                                     w_sb[:, k, m * 128:(m + 1) * 128],
                                     h[:, k, :],
                                     start=False, stop=(last and k == KCH - 1),
                                     skip_group_check=True)

        def cell(ga, gb, c_prev, l):
            sig_fio = spool.tile([128, 12, BL], F32, tag=f"sfio{l}")
            nc.scalar.activation(sig_fio[:], ga[:], AF.Sigmoid)
            tanh_g = spool.tile([128, KCH, BL], F32, tag=f"tg{l}")
            nc.scalar.activation(tanh_g[:], gb[:], AF.Tanh)
            ig = spool.tile([128, KCH, BL], F32, tag=f"ig{l}")
            nc.vector.tensor_mul(ig[:], sig_fio[:, 4:8, :], tanh_g[:])
            c_new = hpool.tile([128, KCH, BL], F32, tag=f"c{l}")
            if c_prev is None:
                nc.vector.tensor_copy(c_new[:], ig[:])
            else:
                # fc on the otherwise-idle Pool engine (all-SBUF operands)
                # so it runs concurrently with ig on DVE.
                fc = spool.tile([128, KCH, BL], F32, tag=f"fc{l}")
                nc.gpsimd.tensor_mul(fc[:], sig_fio[:, 0:4, :], c_prev[:])
                nc.vector.tensor_add(c_new[:], fc[:], ig[:])
            tanh_c = spool.tile([128, KCH, BL], F32, tag=f"tc{l}")
            nc.scalar.activation(tanh_c[:], c_new[:], AF.Tanh)
            h_new = hpool.tile([128, KCH, BL], BF16, tag=f"h{l}")
            nc.vector.tensor_mul(h_new[:], sig_fio[:, 8:12, :], tanh_c[:])
            return c_new, h_new

        def alloc_g1(step):
            # Layer-1 gate PSUM with bias already in place via 16 K=1
            # matmuls (N cols stream regardless of K, so this costs 2048
            # PE cycles/step, ~7%). An engine-side PSUM preload was tried
            # and measured SLOWER: with bufs=1 PSUM the preload can only
            # start after the previous step's activations consume the
            # tile, then queues behind the cell's elementwise work, and
            # whh1 stalled ~0.5us every step waiting for it.
            ga1 = gpool.tile([128, 12, BL], F32, tag="ga1")
            gb1 = gpool.tile([128, KCH, BL], F32, tag="gb1")
            emit_bias1(ga1, gb1, only=False)
            return ga1, gb1

        n_steps = n_tf + n_ar
        h0 = h1 = c0 = c1 = None
        dout = None
        ga0 = gb0 = ga1 = gb1 = None
        for t in range(n_steps):
            if t == 0:
                ga0 = gpool.tile([128, 12, BL], F32, tag="ga0")
                gb0 = gpool.tile([128, KCH, BL], F32, tag="gb0")
                emit_xa(ga0, gb0, x_sb[:, 0, :], only=True)
            c0, h0 = cell(ga0, gb0, c0, 0)
            if t == 0:
                ga1, gb1 = alloc_g1(0)
            else:
                emit_rec(whh1_sb, h1, ga1, gb1, last=False)
            emit_rec(wih1_sb, h0, ga1, gb1, last=True)
            c1, h1 = cell(ga1, gb1, c1, 1)

            # In AR steps, issue the next step's bias matmuls BEFORE the
            # decoder: they only need the (already-consumed) layer-1 gate
            # PSUM, so they fill part of PE's wait for h1. The decoder
            # PSUM aliases gb0's slot (its bank-zeroing start flag is
            # neutralized by xa's own m%4==0 start pattern next step).
            next_is_ar = n_tf <= t + 1 < n_steps
            if next_is_ar:
                ga1n, gb1n = alloc_g1(t + 1)

            if t >= n_tf - 1:
                j = t - (n_tf - 1)
                dec_ps = gpool.tile([FD, BL], F32, tag="gb0")
                for k in range(KCH):
                    nc.tensor.matmul(dec_ps[:], wdec_sb[:, k, :], h1[:, k, :],
                                     start=(k == 0), stop=(k == KCH - 1))
                dout = dpool.tile([FD, BL], BF16, tag="dout")
                nc.scalar.add(dout[:], dec_ps[:], bdec_sb[:])
                nc.sync.dma_start(y_d[j], dout[0:F, :])

            if t + 1 < n_steps:
                ga0 = gpool.tile([128, 12, BL], F32, tag="ga0")
                gb0 = gpool.tile([128, KCH, BL], F32, tag="gb0")
                rhs = x_sb[:, t + 1, :] if t + 1 < n_tf else dout[0:KX, :]
                emit_xa(ga0, gb0, rhs, only=False)
                emit_rec(whh0_sb, h0, ga0, gb0, last=True)
                ga1, gb1 = (ga1n, gb1n) if next_is_ar else alloc_g1(t + 1)

    nc.compile()
    return nc


def _get_program(n_tf=S, n_ar=PRED_LEN - 1):
    key = (n_tf, n_ar)
    with _cache_lock:
        if key not in _cache:
            _cache[key] = _build_program(n_tf, n_ar)
        return _cache[key]


def _kmajor(w):
    """[H, N] -> [128, KCH, N]: row h = k*128 + p lands at [p, k, :]."""
    n = w.shape[1]
    return np.ascontiguousarray(
        w.reshape(KCH, 128, n).transpose(1, 0, 2)).astype(BF16NP)


def _prep_weights(W_enc, b_enc, Wih0, Whh0, bih0, bhh0,
                  Wih1, Whh1, bih1, bhh1, W_dec, b_dec):
    perm = _gate_perm()
    f32 = np.float32

    M0 = (Wih0 @ W_enc)[perm]                                   # [G, F]
    b0 = (Wih0 @ b_enc + bih0 + bhh0)[perm]                     # [G]
    m0t = np.concatenate([M0.T, b0[None, :]], axis=0)           # [KX, G]

    b1p = (bih1 + bhh1)[perm]                                   # [G]
    wdec_aug = np.concatenate(
        [W_dec.T, np.zeros((H, 2), f32)], axis=1)               # [H, FD]
    bdec = np.concatenate([b_dec, np.ones((1,), f32), np.zeros((1,), f32)])

    return {
        "m0t": np.ascontiguousarray(m0t).astype(BF16NP),
        "whh0t": _kmajor(np.ascontiguousarray(Whh0[perm].T)),
        "wih1t": _kmajor(np.ascontiguousarray(Wih1[perm].T)),
        "whh1t": _kmajor(np.ascontiguousarray(Whh1[perm].T)),
        "b1": b1p[None, :].astype(BF16NP),
        "b1bc": np.ascontiguousarray(np.broadcast_to(
            b1p.reshape(NCH, 128).T[:, :, None], (128, NCH, BL))).astype(BF16NP),
        "ones": np.ones((1, BL), BF16NP),
        "wdect": _kmajor(wdec_aug),
        "bdec": np.ascontiguousarray(bdec[:, None], f32),
    }


def _make_in_maps(x, weights, _n_tf=S):
    in_maps = []
    for c in range(NCORES):
        xs = x[c * BL:(c + 1) * BL, :_n_tf, :]                # [BL, n_tf, F]
        xT = xs.transpose(2, 1, 0)                            # [F, n_tf, BL]
        xa = np.concatenate(
            [xT, np.ones((1, _n_tf, BL), np.float32)], axis=0)  # [KX, n_tf, BL]
        in_maps.append(
            {"xT": np.ascontiguousarray(xa).astype(BF16NP), **weights})
    return in_maps


def kernel(x, W_enc, b_enc, Wih0, Whh0, bih0, bhh0,
           Wih1, Whh1, bih1, bhh1, W_dec, b_dec, _n_tf=S, _n_ar=PRED_LEN - 1):
    from concourse.bass_utils import run_bass_kernel_spmd

    x = np.asarray(x, np.float32)
    weights = _prep_weights(
        np.asarray(W_enc, np.float32), np.asarray(b_enc, np.float32),
        np.asarray(Wih0, np.float32), np.asarray(Whh0, np.float32),
        np.asarray(bih0, np.float32), np.asarray(bhh0, np.float32),
        np.asarray(Wih1, np.float32), np.asarray(Whh1, np.float32),
        np.asarray(bih1, np.float32), np.asarray(bhh1, np.float32),
        np.asarray(W_dec, np.float32), np.asarray(b_dec, np.float32))

    nc = _get_program(_n_tf, _n_ar)
    in_maps = _make_in_maps(x, weights, _n_tf)
    res = run_bass_kernel_spmd(nc, in_maps, core_ids=list(range(NCORES)))

    out = np.empty((B, _n_ar + 1, F), np.float32)
    for c in range(NCORES):
        y = np.asarray(res.results[c]["y"], dtype=np.float32)  # [n_out, F, BL]
        out[c * BL:(c + 1) * BL] = y.transpose(2, 0, 1)
    return out


# revision 17
# speedup vs baseline: 1.2683x; 1.2683x over previous
"""Trainium2 Bass kernel for a 2-layer LSTM encoder/decoder forecaster.

Model (per batch element):
  teacher-forced over S=168 steps:  enc -> LSTM0 -> LSTM1 (keep last out)
  autoregressive rollout for 23 more steps feeding decoder output back.

Sharding: data-parallel, batch 1024 -> 8 cores x 128. All weights are
replicated and SBUF-resident; zero inter-core communication.

Layout: everything is FEATURE-MAJOR. Gates are computed as 16 chunks of
[128 gate-rows x 128 batch] PSUM tiles, with the (transposed, chunked)
weights as the stationary matmul operand and h / x as the moving
operand. Because the cell elementwise output h = sig(o)*tanh(c) is then
produced directly in [feature, batch] layout, it is ALREADY the k-chunk
operand the next step's recurrent matmuls need - no PE transposes, no
PSUM->SBUF copies of state anywhere in the loop.

Everything is bf16 on the matmul path (full PE rate at any width, and a
validated ~3e-3 end-to-end error vs the 2e-2 budget; fp8 was measured at
4.3e-2 and rejected). The cell state c stays fp32.

Gate chunk order after host-side row permutation: (f, i, o, g) so that
GA = [f|i] (chunks 0..7) and GB = [o|g] (chunks 8..15), letting each
activation read one contiguous PSUM span.

The encoder is algebraically fused into layer 0 (M0 = Wih0 @ W_enc, bias
folded into an appended ones-row of the feature-major input). Layer 1's
bias enters via 16 K=1 matmuls against a resident ones vector. The
decoder is augmented with a column that regenerates the ones-row so the
AR feedback tile needs no fixup at all: the decoder's SBUF output IS the
next step's input operand.

PE order per steady-state step: [xa(t), whh0(t), bias1(t)] prefetched at
the end of step t-1, then whh1(t), wih1(t) - chosen so every matmul's
input is ready before PE reaches it, keeping PE (the bottleneck at ~28.7k
cycles/step) gapless and at full p-state.
"""

import sys
import threading

sys.path.insert(0, "/opt/trn_rl_repo")

import numpy as np
import ml_dtypes

PRED_LEN = 24
F, I, H = 64, 128, 512
B, S = 1024, 168
NCORES = 8
BL = B // NCORES          # batch per core = 128
G = 4 * H                 # gate width 2048
NCH = G // 128            # 16 gate chunks
KCH = H // 128            # 4 k-chunks of the hidden dim
KX = F + 1                # x operand rows incl. ones row = 65
FD = F + 2                # decoder rows: 64 outputs + ones + pad = 66

BF16NP = ml_dtypes.bfloat16

_cache = {}
_cache_lock = threading.Lock()


def _gate_perm():
    # pytorch gate order i,f,g,o -> reorder rows to (f,i,o,g): chunks
    # 0-3=f, 4-7=i (-> GA), 8-11=o, 12-15=g (-> GB).
    return np.concatenate([
        np.arange(H, 2 * H),        # f
        np.arange(0, H),            # i
        np.arange(3 * H, 4 * H),    # o
        np.arange(2 * H, 3 * H),    # g
    ])


def _build_program(n_tf=S, n_ar=PRED_LEN - 1):
    import concourse.bacc as bacc
    import concourse.tile as tile
    import concourse.mybir as mybir

    F32 = mybir.dt.float32
    BF16 = mybir.dt.bfloat16
    AF = mybir.ActivationFunctionType

    nc = bacc.Bacc("TRN2", target_bir_lowering=False, debug=False,
                   num_devices=NCORES)

    x_d = nc.dram_tensor("xT", [KX, n_tf, BL], BF16, kind="ExternalInput").ap()
    m0_d = nc.dram_tensor("m0t", [KX, G], BF16, kind="ExternalInput").ap()
    whh0_d = nc.dram_tensor("whh0t", [128, KCH, G], BF16, kind="ExternalInput").ap()
    wih1_d = nc.dram_tensor("wih1t", [128, KCH, G], BF16, kind="ExternalInput").ap()
    whh1_d = nc.dram_tensor("whh1t", [128, KCH, G], BF16, kind="ExternalInput").ap()
    b1_d = nc.dram_tensor("b1", [1, G], BF16, kind="ExternalInput").ap()
    b1bc_d = nc.dram_tensor("b1bc", [128, NCH, BL], BF16, kind="ExternalInput").ap()
    ones_d = nc.dram_tensor("ones", [1, BL], BF16, kind="ExternalInput").ap()
    wdec_d = nc.dram_tensor("wdect", [128, KCH, FD], BF16, kind="ExternalInput").ap()
    bdec_d = nc.dram_tensor("bdec", [FD, 1], F32, kind="ExternalInput").ap()
    y_d = nc.dram_tensor("y", [n_ar + 1, F, BL], BF16, kind="ExternalOutput").ap()

    from contextlib import ExitStack
    with tile.TileContext(nc) as tc, ExitStack() as ctx:
        wpool = ctx.enter_context(tc.tile_pool(name="w", bufs=1))
        spool = ctx.enter_context(tc.tile_pool(name="s", bufs=2))
        hpool = ctx.enter_context(tc.tile_pool(name="h", bufs=2))
        dpool = ctx.enter_context(tc.tile_pool(name="d", bufs=2))
        gpool = ctx.enter_context(tc.tile_pool(name="g", bufs=1, space="PSUM"))

        # ---- resident weights + input ----
        # split the x load so step 0 only waits on the first few columns
        x_sb = wpool.tile([KX, n_tf, BL], BF16)
        x_head = min(8, n_tf)
        nc.sync.dma_start(x_sb[:, 0:x_head, :], x_d[:, 0:x_head, :])
        if x_head < n_tf:
            nc.sync.dma_start(x_sb[:, x_head:, :], x_d[:, x_head:, :])
        m0_sb = wpool.tile([KX, G], BF16)
        nc.sync.dma_start(m0_sb[:], m0_d[:])
        whh0_sb = wpool.tile([128, KCH, G], BF16)
        nc.sync.dma_start(whh0_sb[:], whh0_d[:])
        wih1_sb = wpool.tile([128, KCH, G], BF16)
        nc.sync.dma_start(wih1_sb[:], wih1_d[:])
        whh1_sb = wpool.tile([128, KCH, G], BF16)
        nc.sync.dma_start(whh1_sb[:], whh1_d[:])
        b1_sb = wpool.tile([1, G], BF16)
        nc.sync.dma_start(b1_sb[:], b1_d[:])
        b1bc_sb = wpool.tile([128, NCH, BL], BF16)
        nc.sync.dma_start(b1bc_sb[:], b1bc_d[:])
        ones_sb = wpool.tile([1, BL], BF16)
        nc.sync.dma_start(ones_sb[:], ones_d[:])
        wdec_sb = wpool.tile([128, KCH, FD], BF16)
        nc.sync.dma_start(wdec_sb[:], wdec_d[:])
        bdec_sb = wpool.tile([FD, 1], F32)
        nc.sync.dma_start(bdec_sb[:], bdec_d[:])

        # ga holds chunks 0-11 = f|i|o (3 PSUM banks), gb holds 12-15 = g
        # (1 bank): one sigmoid + one tanh cover all gate activations.
        def halves(ga, gb, m):
            return ga[:, m, :] if m < 12 else gb[:, m - 12, :]

        # start=True lazily zeroes the WHOLE 2KB PSUM bank (4 of our 512B
        # chunk regions), so only the first write per bank may set it; the
        # sibling regions still see the bank's pending-zero and overwrite.
        def emit_xa(ga, gb, rhs, only):
            # input-side gate contribution; `only`=True closes the group
            # (t=0 has no recurrent term).
            for m in range(NCH):
                nc.tensor.matmul(halves(ga, gb, m),
                                 m0_sb[:, m * 128:(m + 1) * 128], rhs,
                                 start=(m % 4 == 0), stop=only,
                                 skip_group_check=True)

        def emit_bias1(ga, gb, only):
            for m in range(NCH):
                nc.tensor.matmul(halves(ga, gb, m),
                                 b1_sb[:, m * 128:(m + 1) * 128], ones_sb[:],
                                 start=(m % 4 == 0), stop=only,
                                 skip_group_check=True)

        def emit_rec(w_sb, h, ga, gb, last):
            for m in range(NCH):
                out = halves(ga, gb, m)
                for k in range(KCH):
                    nc.tensor.matmul(out,
                                     w_sb[:, k, m * 128:(m + 1) * 128],
                                     h[:, k, :],
                                     start=False, stop=(last and k == KCH - 1),
                                     skip_group_check=True)

        def cell(ga, gb, c_prev, l):
            sig_fio = spool.tile([128, 12, BL], F32, tag=f"sfio{l}")
            nc.scalar.activation(sig_fio[:], ga[:], AF.Sigmoid)
            tanh_g = spool.tile([128, KCH, BL], F32, tag=f"tg{l}")
            nc.scalar.activation(tanh_g[:], gb[:], AF.Tanh)
            ig = spool.tile([128, KCH, BL], F32, tag=f"ig{l}")
            nc.vector.tensor_mul(ig[:], sig_fio[:, 4:8, :], tanh_g[:])
            c_new = hpool.tile([128, KCH, BL], F32, tag=f"c{l}")
            if c_prev is None:
                nc.vector.tensor_copy(c_new[:], ig[:])
            else:
                # fc stays on DVE: gpsimd was measured ~5us/op (Q7 DSP),
                # poisoning the cell's critical chain.
                fc = spool.tile([128, KCH, BL], F32, tag=f"fc{l}")
                nc.vector.tensor_mul(fc[:], sig_fio[:, 0:4, :], c_prev[:])
                nc.vector.tensor_add(c_new[:], fc[:], ig[:])
            tanh_c = spool.tile([128, KCH, BL], F32, tag=f"tc{l}")
            nc.scalar.activation(tanh_c[:], c_new[:], AF.Tanh)
            h_new = hpool.tile([128, KCH, BL], BF16, tag=f"h{l}")
            nc.vector.tensor_mul(h_new[:], sig_fio[:, 8:12, :], tanh_c[:])
            return c_new, h_new

        def alloc_g1(step):
            # Layer-1 gate PSUM with bias already in place via 16 K=1
            # matmuls (N cols stream regardless of K, so this costs 2048
            # PE cycles/step, ~7%). An engine-side PSUM preload was tried
            # and measured SLOWER: with bufs=1 PSUM the preload can only
            # start after the previous step's activations consume the
            # tile, then queues behind the cell's elementwise work, and
            # whh1 stalled ~0.5us every step waiting for it.
            ga1 = gpool.tile([128, 12, BL], F32, tag="ga1")
            gb1 = gpool.tile([128, KCH, BL], F32, tag="gb1")
            emit_bias1(ga1, gb1, only=False)
            return ga1, gb1

        n_steps = n_tf + n_ar
        h0 = h1 = c0 = c1 = None
        dout = None
        ga0 = gb0 = ga1 = gb1 = None
        for t in range(n_steps):
            if t == 0:
                ga0 = gpool.tile([128, 12, BL], F32, tag="ga0")
                gb0 = gpool.tile([128, KCH, BL], F32, tag="gb0")
                emit_xa(ga0, gb0, x_sb[:, 0, :], only=True)
            c0, h0 = cell(ga0, gb0, c0, 0)
            if t == 0:
                ga1, gb1 = alloc_g1(0)
            else:
                emit_rec(whh1_sb, h1, ga1, gb1, last=False)
            emit_rec(wih1_sb, h0, ga1, gb1, last=True)
            c1, h1 = cell(ga1, gb1, c1, 1)

            # In AR steps, issue the next step's bias matmuls BEFORE the
            # decoder: they only need the (already-consumed) layer-1 gate
            # PSUM, so they fill part of PE's wait for h1. The decoder
            # PSUM aliases gb0's slot (its bank-zeroing start flag is
            # neutralized by xa's own m%4==0 start pattern next step).
            next_is_ar = n_tf <= t + 1 < n_steps
            if next_is_ar:
                ga1n, gb1n = alloc_g1(t + 1)

            if t >= n_tf - 1:
                j = t - (n_tf - 1)
                dec_ps = gpool.tile([FD, BL], F32, tag="gb0")
                for k in range(KCH):
                    nc.tensor.matmul(dec_ps[:], wdec_sb[:, k, :], h1[:, k, :],
                                     start=(k == 0), stop=(k == KCH - 1))
                dout = dpool.tile([FD, BL], BF16, tag="dout")
                nc.scalar.add(dout[:], dec_ps[:], bdec_sb[:])
                nc.sync.dma_start(y_d[j], dout[0:F, :])

            if t + 1 < n_steps:
                ga0 = gpool.tile([128, 12, BL], F32, tag="ga0")
                gb0 = gpool.tile([128, KCH, BL], F32, tag="gb0")
                rhs = x_sb[:, t + 1, :] if t + 1 < n_tf else dout[0:KX, :]
                emit_xa(ga0, gb0, rhs, only=False)
                emit_rec(whh0_sb, h0, ga0, gb0, last=True)
                ga1, gb1 = (ga1n, gb1n) if next_is_ar else alloc_g1(t + 1)

    nc.compile()
    return nc


def _get_program(n_tf=S, n_ar=PRED_LEN - 1):
    key = (n_tf, n_ar)
    with _cache_lock:
        if key not in _cache:
            _cache[key] = _build_program(n_tf, n_ar)
        return _cache[key]


def _kmajor(w):
    """[H, N] -> [128, KCH, N]: row h = k*128 + p lands at [p, k, :]."""
    n = w.shape[1]
    return np.ascontiguousarray(
        w.reshape(KCH, 128, n).transpose(1, 0, 2)).astype(BF16NP)


def _prep_weights(W_enc, b_enc, Wih0, Whh0, bih0, bhh0,
                  Wih1, Whh1, bih1, bhh1, W_dec, b_dec):
    perm = _gate_perm()
    f32 = np.float32

    M0 = (Wih0 @ W_enc)[perm]                                   # [G, F]
    b0 = (Wih0 @ b_enc + bih0 + bhh0)[perm]                     # [G]
    m0t = np.concatenate([M0.T, b0[None, :]], axis=0)           # [KX, G]

    b1p = (bih1 + bhh1)[perm]                                   # [G]
    wdec_aug = np.concatenate(
        [W_dec.T, np.zeros((H, 2), f32)], axis=1)               # [H, FD]
    bdec = np.concatenate([b_dec, np.ones((1,), f32), np.zeros((1,), f32)])

    return {
        "m0t": np.ascontiguousarray(m0t).astype(BF16NP),
        "whh0t": _kmajor(np.ascontiguousarray(Whh0[perm].T)),
        "wih1t": _kmajor(np.ascontiguousarray(Wih1[perm].T)),
        "whh1t": _kmajor(np.ascontiguousarray(Whh1[perm].T)),
        "b1": b1p[None, :].astype(BF16NP),
        "b1bc": np.ascontiguousarray(np.broadcast_to(
            b1p.reshape(NCH, 128).T[:, :, None], (128, NCH, BL))).astype(BF16NP),
        "ones": np.ones((1, BL), BF16NP),
        "wdect": _kmajor(wdec_aug),
        "bdec": np.ascontiguousarray(bdec[:, None], f32),
    }


def _make_in_maps(x, weights, _n_tf=S):
    in_maps = []
    for c in range(NCORES):
        xs = x[c * BL:(c + 1) * BL, :_n_tf, :]                # [BL, n_tf, F]
        xT = xs.transpose(2, 1, 0)                            # [F, n_tf, BL]
        xa = np.concatenate(
            [xT, np.ones((1, _n_tf, BL), np.float32)], axis=0)  # [KX, n_tf, BL]
        in_maps.append(
            {"xT": np.ascontiguousarray(xa).astype(BF16NP), **weights})
    return in_maps


def kernel(x, W_enc, b_enc, Wih0, Whh0, bih0, bhh0,
           Wih1, Whh1, bih1, bhh1, W_dec, b_dec, _n_tf=S, _n_ar=PRED_LEN - 1):
    from concourse.bass_utils import run_bass_kernel_spmd

    x = np.asarray(x, np.float32)
    weights = _prep_weights(
        np.asarray(W_enc, np.float32), np.asarray(b_enc, np.float32),
        np.asarray(Wih0, np.float32), np.asarray(Whh0, np.float32),
        np.asarray(bih0, np.float32), np.asarray(bhh0, np.float32),
        np.asarray(Wih1, np.float32), np.asarray(Whh1, np.float32),
        np.asarray(bih1, np.float32), np.asarray(bhh1, np.float32),
        np.asarray(W_dec, np.float32), np.asarray(b_dec, np.float32))

    nc = _get_program(_n_tf, _n_ar)
    in_maps = _make_in_maps(x, weights, _n_tf)
    res = run_bass_kernel_spmd(nc, in_maps, core_ids=list(range(NCORES)))

    out = np.empty((B, _n_ar + 1, F), np.float32)
    for c in range(NCORES):
        y = np.asarray(res.results[c]["y"], dtype=np.float32)  # [n_out, F, BL]
        out[c * BL:(c + 1) * BL] = y.transpose(2, 0, 1)
    return out


# revision 18
# speedup vs baseline: 1.3161x; 1.0376x over previous
"""Trainium2 Bass kernel for a 2-layer LSTM encoder/decoder forecaster.

Model (per batch element):
  teacher-forced over S=168 steps:  enc -> LSTM0 -> LSTM1 (keep last out)
  autoregressive rollout for 23 more steps feeding decoder output back.

Sharding: data-parallel, batch 1024 -> 8 cores x 128. All weights are
replicated and SBUF-resident; zero inter-core communication.

Layout: everything is FEATURE-MAJOR. Gates are computed as 16 chunks of
[128 gate-rows x 128 batch] PSUM tiles, with the (transposed, chunked)
weights as the stationary matmul operand and h / x as the moving
operand. Because the cell elementwise output h = sig(o)*tanh(c) is then
produced directly in [feature, batch] layout, it is ALREADY the k-chunk
operand the next step's recurrent matmuls need - no PE transposes, no
PSUM->SBUF copies of state anywhere in the loop.

Everything is bf16 on the matmul path (full PE rate at any width, and a
validated ~3e-3 end-to-end error vs the 2e-2 budget; fp8 was measured at
4.3e-2 and rejected). The cell state c stays fp32.

Gate chunk order after host-side row permutation: (f, i, o, g) so that
GA = [f|i] (chunks 0..7) and GB = [o|g] (chunks 8..15), letting each
activation read one contiguous PSUM span.

The encoder is algebraically fused into layer 0 (M0 = Wih0 @ W_enc, bias
folded into an appended ones-row of the feature-major input). Layer 1's
bias enters via 16 K=1 matmuls against a resident ones vector. The
decoder is augmented with a column that regenerates the ones-row so the
AR feedback tile needs no fixup at all: the decoder's SBUF output IS the
next step's input operand.

PE order per steady-state step: [xa(t), whh0(t), bias1(t)] prefetched at
the end of step t-1, then whh1(t), wih1(t) - chosen so every matmul's
input is ready before PE reaches it, keeping PE (the bottleneck at ~28.7k
cycles/step) gapless and at full p-state.
"""

import sys
import threading

sys.path.insert(0, "/opt/trn_rl_repo")

import numpy as np
import ml_dtypes

PRED_LEN = 24
F, I, H = 64, 128, 512
B, S = 1024, 168
NCORES = 8
BL = B // NCORES          # batch per core = 128
G = 4 * H                 # gate width 2048
NCH = G // 128            # 16 gate chunks
KCH = H // 128            # 4 k-chunks of the hidden dim
KX = F + 1                # x operand rows incl. ones row = 65
FD = F + 2                # decoder rows: 64 outputs + ones + pad = 66

BF16NP = ml_dtypes.bfloat16

_cache = {}
_cache_lock = threading.Lock()


def _gate_perm():
    # pytorch gate order i,f,g,o -> reorder rows to (f,i,o,g): chunks
    # 0-3=f, 4-7=i (-> GA), 8-11=o, 12-15=g (-> GB).
    return np.concatenate([
        np.arange(H, 2 * H),        # f
        np.arange(0, H),            # i
        np.arange(3 * H, 4 * H),    # o
        np.arange(2 * H, 3 * H),    # g
    ])


def _build_program(n_tf=S, n_ar=PRED_LEN - 1):
    import concourse.bacc as bacc
    import concourse.tile as tile
    import concourse.mybir as mybir

    F32 = mybir.dt.float32
    BF16 = mybir.dt.bfloat16
    AF = mybir.ActivationFunctionType

    nc = bacc.Bacc("TRN2", target_bir_lowering=False, debug=False,
                   num_devices=NCORES)

    x_d = nc.dram_tensor("xT", [KX, n_tf, BL], BF16, kind="ExternalInput").ap()
    m0_d = nc.dram_tensor("m0t", [KX, G], BF16, kind="ExternalInput").ap()
    whh0_d = nc.dram_tensor("whh0t", [128, KCH, G], BF16, kind="ExternalInput").ap()
    wih1_d = nc.dram_tensor("wih1t", [128, KCH, G], BF16, kind="ExternalInput").ap()
    whh1_d = nc.dram_tensor("whh1t", [128, KCH, G], BF16, kind="ExternalInput").ap()
    b1_d = nc.dram_tensor("b1", [1, G], BF16, kind="ExternalInput").ap()
    ones_d = nc.dram_tensor("ones", [1, BL], BF16, kind="ExternalInput").ap()
    wdec_d = nc.dram_tensor("wdect", [128, KCH, FD], BF16, kind="ExternalInput").ap()
    bdec_d = nc.dram_tensor("bdec", [FD, 1], F32, kind="ExternalInput").ap()
    y_d = nc.dram_tensor("y", [n_ar + 1, F, BL], BF16, kind="ExternalOutput").ap()

    from contextlib import ExitStack
    with tile.TileContext(nc) as tc, ExitStack() as ctx:
        wpool = ctx.enter_context(tc.tile_pool(name="w", bufs=1))
        spool = ctx.enter_context(tc.tile_pool(name="s", bufs=2))
        hpool = ctx.enter_context(tc.tile_pool(name="h", bufs=2))
        dpool = ctx.enter_context(tc.tile_pool(name="d", bufs=2))
        gpool = ctx.enter_context(tc.tile_pool(name="g", bufs=1, space="PSUM"))

        # ---- resident weights + input ----
        x_sb = wpool.tile([KX, n_tf, BL], BF16)
        nc.sync.dma_start(x_sb[:], x_d[:])
        m0_sb = wpool.tile([KX, G], BF16)
        nc.sync.dma_start(m0_sb[:], m0_d[:])
        whh0_sb = wpool.tile([128, KCH, G], BF16)
        nc.sync.dma_start(whh0_sb[:], whh0_d[:])
        wih1_sb = wpool.tile([128, KCH, G], BF16)
        nc.sync.dma_start(wih1_sb[:], wih1_d[:])
        whh1_sb = wpool.tile([128, KCH, G], BF16)
        nc.sync.dma_start(whh1_sb[:], whh1_d[:])
        b1_sb = wpool.tile([1, G], BF16)
        nc.sync.dma_start(b1_sb[:], b1_d[:])
        ones_sb = wpool.tile([1, BL], BF16)
        nc.sync.dma_start(ones_sb[:], ones_d[:])
        wdec_sb = wpool.tile([128, KCH, FD], BF16)
        nc.sync.dma_start(wdec_sb[:], wdec_d[:])
        bdec_sb = wpool.tile([FD, 1], F32)
        nc.sync.dma_start(bdec_sb[:], bdec_d[:])

        def halves(ga, gb, m):
            return (ga if m < 8 else gb)[:, m % 8, :]

        # start=True lazily zeroes the WHOLE 2KB PSUM bank (4 of our 512B
        # chunk regions), so only the first write per bank may set it; the
        # sibling regions still see the bank's pending-zero and overwrite.
        def emit_xa(ga, gb, rhs, only):
            # input-side gate contribution; `only`=True closes the group
            # (t=0 has no recurrent term).
            for m in range(NCH):
                nc.tensor.matmul(halves(ga, gb, m),
                                 m0_sb[:, m * 128:(m + 1) * 128], rhs,
                                 start=(m % 4 == 0), stop=only,
                                 skip_group_check=True)

        def emit_bias1(ga, gb, only):
            for m in range(NCH):
                nc.tensor.matmul(halves(ga, gb, m),
                                 b1_sb[:, m * 128:(m + 1) * 128], ones_sb[:],
                                 start=(m % 4 == 0), stop=only,
                                 skip_group_check=True)

        def emit_rec(w_sb, h, ga, gb, last):
            for m in range(NCH):
                out = halves(ga, gb, m)
                for k in range(KCH):
                    nc.tensor.matmul(out,
                                     w_sb[:, k, m * 128:(m + 1) * 128],
                                     h[:, k, :],
                                     start=False, stop=(last and k == KCH - 1),
                                     skip_group_check=True)

        def cell(ga, gb, c_prev, l):
            sig_fi = spool.tile([128, 8, BL], F32, tag=f"sfi{l}")
            nc.scalar.activation(sig_fi[:], ga[:], AF.Sigmoid)
            tanh_g = spool.tile([128, KCH, BL], F32, tag=f"tg{l}")
            nc.scalar.activation(tanh_g[:], gb[:, 4:8, :], AF.Tanh)
            sig_o = spool.tile([128, KCH, BL], F32, tag=f"so{l}")
            nc.scalar.activation(sig_o[:], gb[:, 0:4, :], AF.Sigmoid)
            ig = spool.tile([128, KCH, BL], F32, tag=f"ig{l}")
            nc.vector.tensor_mul(ig[:], sig_fi[:, 4:8, :], tanh_g[:])
            c_new = hpool.tile([128, KCH, BL], F32, tag=f"c{l}")
            if c_prev is None:
                nc.vector.tensor_copy(c_new[:], ig[:])
            else:
                fc = spool.tile([128, KCH, BL], F32, tag=f"fc{l}")
                nc.vector.tensor_mul(fc[:], sig_fi[:, 0:4, :], c_prev[:])
                nc.vector.tensor_add(c_new[:], fc[:], ig[:])
            tanh_c = spool.tile([128, KCH, BL], F32, tag=f"tc{l}")
            nc.scalar.activation(tanh_c[:], c_new[:], AF.Tanh)
            h_new = hpool.tile([128, KCH, BL], BF16, tag=f"h{l}")
            nc.vector.tensor_mul(h_new[:], sig_o[:], tanh_c[:])
            return c_new, h_new

        def alloc_g1(step):
            ga1 = gpool.tile([128, 8, BL], F32, tag="ga1")
            gb1 = gpool.tile([128, 8, BL], F32, tag="gb1")
            emit_bias1(ga1, gb1, only=False)
            return ga1, gb1

        n_steps = n_tf + n_ar
        h0 = h1 = c0 = c1 = None
        dout = None
        ga0 = gb0 = ga1 = gb1 = None
        for t in range(n_steps):
            if t == 0:
                ga0 = gpool.tile([128, 8, BL], F32, tag="ga0")
                gb0 = gpool.tile([128, 8, BL], F32, tag="gb0")
                emit_xa(ga0, gb0, x_sb[:, 0, :], only=True)
            c0, h0 = cell(ga0, gb0, c0, 0)
            if t == 0:
                ga1, gb1 = alloc_g1(0)
            else:
                emit_rec(whh1_sb, h1, ga1, gb1, last=False)
            emit_rec(wih1_sb, h0, ga1, gb1, last=True)
            c1, h1 = cell(ga1, gb1, c1, 1)

            if t >= n_tf - 1:
                j = t - (n_tf - 1)
                dec_ps = gpool.tile([FD, BL], F32, tag="gb1")
                for k in range(KCH):
                    nc.tensor.matmul(dec_ps[:], wdec_sb[:, k, :], h1[:, k, :],
                                     start=(k == 0), stop=(k == KCH - 1))
                dout = dpool.tile([FD, BL], BF16, tag="dout")
                nc.scalar.add(dout[:], dec_ps[:], bdec_sb[:])
                nc.sync.dma_start(y_d[j], dout[0:F, :])

            if t + 1 < n_steps:
                ga0 = gpool.tile([128, 8, BL], F32, tag="ga0")
                gb0 = gpool.tile([128, 8, BL], F32, tag="gb0")
                rhs = x_sb[:, t + 1, :] if t + 1 < n_tf else dout[0:KX, :]
                emit_xa(ga0, gb0, rhs, only=False)
                emit_rec(whh0_sb, h0, ga0, gb0, last=True)
                ga1, gb1 = alloc_g1(t + 1)

    nc.compile()
    return nc


def _get_program(n_tf=S, n_ar=PRED_LEN - 1):
    key = (n_tf, n_ar)
    with _cache_lock:
        if key not in _cache:
            _cache[key] = _build_program(n_tf, n_ar)
        return _cache[key]


def _kmajor(w):
    """[H, N] -> [128, KCH, N]: row h = k*128 + p lands at [p, k, :]."""
    n = w.shape[1]
    return np.ascontiguousarray(
        w.reshape(KCH, 128, n).transpose(1, 0, 2)).astype(BF16NP)


def _prep_weights(W_enc, b_enc, Wih0, Whh0, bih0, bhh0,
                  Wih1, Whh1, bih1, bhh1, W_dec, b_dec):
    perm = _gate_perm()
    f32 = np.float32

    M0 = (Wih0 @ W_enc)[perm]                                   # [G, F]
    b0 = (Wih0 @ b_enc + bih0 + bhh0)[perm]                     # [G]
    m0t = np.concatenate([M0.T, b0[None, :]], axis=0)           # [KX, G]

    b1p = (bih1 + bhh1)[perm]                                   # [G]
    wdec_aug = np.concatenate(
        [W_dec.T, np.zeros((H, 2), f32)], axis=1)               # [H, FD]
    bdec = np.concatenate([b_dec, np.ones((1,), f32), np.zeros((1,), f32)])

    return {
        "m0t": np.ascontiguousarray(m0t).astype(BF16NP),
        "whh0t": _kmajor(np.ascontiguousarray(Whh0[perm].T)),
        "wih1t": _kmajor(np.ascontiguousarray(Wih1[perm].T)),
        "whh1t": _kmajor(np.ascontiguousarray(Whh1[perm].T)),
        "b1": b1p[None, :].astype(BF16NP),
        "ones": np.ones((1, BL), BF16NP),
        "wdect": _kmajor(wdec_aug),
        "bdec": np.ascontiguousarray(bdec[:, None], f32),
    }


def _make_in_maps(x, weights, _n_tf=S):
    in_maps = []
    for c in range(NCORES):
        xs = x[c * BL:(c + 1) * BL, :_n_tf, :]                # [BL, n_tf, F]
        xT = xs.transpose(2, 1, 0)                            # [F, n_tf, BL]
        xa = np.concatenate(
            [xT, np.ones((1, _n_tf, BL), np.float32)], axis=0)  # [KX, n_tf, BL]
        in_maps.append(
            {"xT": np.ascontiguousarray(xa).astype(BF16NP), **weights})
    return in_maps


def kernel(x, W_enc, b_enc, Wih0, Whh0, bih0, bhh0,
           Wih1, Whh1, bih1, bhh1, W_dec, b_dec, _n_tf=S, _n_ar=PRED_LEN - 1):
    from concourse.bass_utils import run_bass_kernel_spmd

    x = np.asarray(x, np.float32)
    weights = _prep_weights(
        np.asarray(W_enc, np.float32), np.asarray(b_enc, np.float32),
        np.asarray(Wih0, np.float32), np.asarray(Whh0, np.float32),
        np.asarray(bih0, np.float32), np.asarray(bhh0, np.float32),
        np.asarray(Wih1, np.float32), np.asarray(Whh1, np.float32),
        np.asarray(bih1, np.float32), np.asarray(bhh1, np.float32),
        np.asarray(W_dec, np.float32), np.asarray(b_dec, np.float32))

    nc = _get_program(_n_tf, _n_ar)
    in_maps = _make_in_maps(x, weights, _n_tf)
    res = run_bass_kernel_spmd(nc, in_maps, core_ids=list(range(NCORES)))

    out = np.empty((B, _n_ar + 1, F), np.float32)
    for c in range(NCORES):
        y = np.asarray(res.results[c]["y"], dtype=np.float32)  # [n_out, F, BL]
        out[c * BL:(c + 1) * BL] = y.transpose(2, 0, 1)
    return out


# revision 21
# speedup vs baseline: 1.3222x; 1.0047x over previous
"""Trainium2 Bass kernel for a 2-layer LSTM encoder/decoder forecaster.

Model (per batch element):
  teacher-forced over S=168 steps:  enc -> LSTM0 -> LSTM1 (keep last out)
  autoregressive rollout for 23 more steps feeding decoder output back.

Sharding: data-parallel, batch 1024 -> 8 cores x 128. All weights are
replicated and SBUF-resident; zero inter-core communication.

Layout: everything is FEATURE-MAJOR. Gates are computed as 16 chunks of
[128 gate-rows x 128 batch] PSUM tiles, with the (transposed, chunked)
weights as the stationary matmul operand and h / x as the moving
operand. Because the cell elementwise output h = sig(o)*tanh(c) is then
produced directly in [feature, batch] layout, it is ALREADY the k-chunk
operand the next step's recurrent matmuls need - no PE transposes, no
PSUM->SBUF copies of state anywhere in the loop.

Everything is bf16 on the matmul path (full PE rate at any width, and a
validated ~3e-3 end-to-end error vs the 2e-2 budget; fp8 was measured at
4.3e-2 and rejected). The cell state c stays fp32.

Gate chunk order after host-side row permutation: (f, i, o, g) so that
GA = [f|i] (chunks 0..7) and GB = [o|g] (chunks 8..15), letting each
activation read one contiguous PSUM span.

The encoder is algebraically fused into layer 0 (M0 = Wih0 @ W_enc, bias
folded into an appended ones-row of the feature-major input). Layer 1's
bias enters via 16 K=1 matmuls against a resident ones vector. The
decoder is augmented with a column that regenerates the ones-row so the
AR feedback tile needs no fixup at all: the decoder's SBUF output IS the
next step's input operand.

PE order per steady-state step: [xa(t), whh0(t), bias1(t)] prefetched at
the end of step t-1, then whh1(t), wih1(t) - chosen so every matmul's
input is ready before PE reaches it, keeping PE (the bottleneck at ~28.7k
cycles/step) gapless and at full p-state.
"""

import sys
import threading

sys.path.insert(0, "/opt/trn_rl_repo")

import numpy as np
import ml_dtypes

PRED_LEN = 24
F, I, H = 64, 128, 512
B, S = 1024, 168
NCORES = 8
BL = B // NCORES          # batch per core = 128
G = 4 * H                 # gate width 2048
NCH = G // 128            # 16 gate chunks
KCH = H // 128            # 4 k-chunks of the hidden dim
KX = F + 1                # x operand rows incl. ones row = 65
FD = F + 2                # decoder rows: 64 outputs + ones + pad = 66

BF16NP = ml_dtypes.bfloat16

_cache = {}
_cache_lock = threading.Lock()


def _gate_perm():
    # pytorch gate order i,f,g,o -> reorder rows to (f,i,o,g): chunks
    # 0-3=f, 4-7=i (-> GA), 8-11=o, 12-15=g (-> GB).
    return np.concatenate([
        np.arange(H, 2 * H),        # f
        np.arange(0, H),            # i
        np.arange(3 * H, 4 * H),    # o
        np.arange(2 * H, 3 * H),    # g
    ])


def _build_program(n_tf=S, n_ar=PRED_LEN - 1):
    import concourse.bacc as bacc
    import concourse.tile as tile
    import concourse.mybir as mybir

    F32 = mybir.dt.float32
    BF16 = mybir.dt.bfloat16
    AF = mybir.ActivationFunctionType

    nc = bacc.Bacc("TRN2", target_bir_lowering=False, debug=False,
                   num_devices=NCORES)

    x_d = nc.dram_tensor("xT", [KX, n_tf, BL], BF16, kind="ExternalInput").ap()
    m0_d = nc.dram_tensor("m0t", [KX, G], BF16, kind="ExternalInput").ap()
    whh0_d = nc.dram_tensor("whh0t", [128, KCH, G], BF16, kind="ExternalInput").ap()
    wih1_d = nc.dram_tensor("wih1t", [128, KCH, G], BF16, kind="ExternalInput").ap()
    whh1_d = nc.dram_tensor("whh1t", [128, KCH, G], BF16, kind="ExternalInput").ap()
    b1_d = nc.dram_tensor("b1", [1, G], BF16, kind="ExternalInput").ap()
    ones_d = nc.dram_tensor("ones", [1, BL], BF16, kind="ExternalInput").ap()
    wdec_d = nc.dram_tensor("wdect", [128, KCH, FD], BF16, kind="ExternalInput").ap()
    bdec_d = nc.dram_tensor("bdec", [FD, 1], F32, kind="ExternalInput").ap()
    y_d = nc.dram_tensor("y", [n_ar + 1, F, BL], BF16, kind="ExternalOutput").ap()

    from contextlib import ExitStack
    with tile.TileContext(nc) as tc, ExitStack() as ctx:
        wpool = ctx.enter_context(tc.tile_pool(name="w", bufs=1))
        spool = ctx.enter_context(tc.tile_pool(name="s", bufs=2))
        hpool = ctx.enter_context(tc.tile_pool(name="h", bufs=2))
        dpool = ctx.enter_context(tc.tile_pool(name="d", bufs=2))
        gpool = ctx.enter_context(tc.tile_pool(name="g", bufs=1, space="PSUM"))

        # ---- resident weights + input ----
        # split the x load so step 0 only waits on the first few columns
        x_sb = wpool.tile([KX, n_tf, BL], BF16)
        x_head = min(8, n_tf)
        nc.sync.dma_start(x_sb[:, 0:x_head, :], x_d[:, 0:x_head, :])
        if x_head < n_tf:
            nc.sync.dma_start(x_sb[:, x_head:, :], x_d[:, x_head:, :])
        m0_sb = wpool.tile([KX, G], BF16)
        nc.sync.dma_start(m0_sb[:], m0_d[:])
        whh0_sb = wpool.tile([128, KCH, G], BF16)
        nc.sync.dma_start(whh0_sb[:], whh0_d[:])
        wih1_sb = wpool.tile([128, KCH, G], BF16)
        nc.sync.dma_start(wih1_sb[:], wih1_d[:])
        whh1_sb = wpool.tile([128, KCH, G], BF16)
        nc.sync.dma_start(whh1_sb[:], whh1_d[:])
        b1_sb = wpool.tile([1, G], BF16)
        nc.sync.dma_start(b1_sb[:], b1_d[:])
        ones_sb = wpool.tile([1, BL], BF16)
        nc.sync.dma_start(ones_sb[:], ones_d[:])
        wdec_sb = wpool.tile([128, KCH, FD], BF16)
        nc.sync.dma_start(wdec_sb[:], wdec_d[:])
        bdec_sb = wpool.tile([FD, 1], F32)
        nc.sync.dma_start(bdec_sb[:], bdec_d[:])

        def halves(ga, gb, m):
            return (ga if m < 8 else gb)[:, m % 8, :]

        # start=True lazily zeroes the WHOLE 2KB PSUM bank (4 of our 512B
        # chunk regions), so only the first write per bank may set it; the
        # sibling regions still see the bank's pending-zero and overwrite.
        def emit_xa(ga, gb, rhs, only):
            # input-side gate contribution; `only`=True closes the group
            # (t=0 has no recurrent term).
            for m in range(NCH):
                nc.tensor.matmul(halves(ga, gb, m),
                                 m0_sb[:, m * 128:(m + 1) * 128], rhs,
                                 start=(m % 4 == 0), stop=only,
                                 skip_group_check=True)

        def emit_bias1(ga, gb, only):
            for m in range(NCH):
                nc.tensor.matmul(halves(ga, gb, m),
                                 b1_sb[:, m * 128:(m + 1) * 128], ones_sb[:],
                                 start=(m % 4 == 0), stop=only,
                                 skip_group_check=True)

        def emit_rec(w_sb, h, ga, gb, last):
            for m in range(NCH):
                out = halves(ga, gb, m)
                for k in range(KCH):
                    nc.tensor.matmul(out,
                                     w_sb[:, k, m * 128:(m + 1) * 128],
                                     h[:, k, :],
                                     start=False, stop=(last and k == KCH - 1),
                                     skip_group_check=True)

        def cell(ga, gb, c_prev, l):
            sig_fi = spool.tile([128, 8, BL], F32, tag=f"sfi{l}")
            nc.scalar.activation(sig_fi[:], ga[:], AF.Sigmoid)
            tanh_g = spool.tile([128, KCH, BL], F32, tag=f"tg{l}")
            nc.scalar.activation(tanh_g[:], gb[:, 4:8, :], AF.Tanh)
            sig_o = spool.tile([128, KCH, BL], F32, tag=f"so{l}")
            nc.scalar.activation(sig_o[:], gb[:, 0:4, :], AF.Sigmoid)
            # fc first on the DVE queue: its input sig_fi is ready one Act
            # op earlier than ig's tanh_g, shortening the serial cell chain.
            c_new = hpool.tile([128, KCH, BL], F32, tag=f"c{l}")
            ig = spool.tile([128, KCH, BL], F32, tag=f"ig{l}")
            if c_prev is None:
                nc.vector.tensor_mul(ig[:], sig_fi[:, 4:8, :], tanh_g[:])
                nc.vector.tensor_copy(c_new[:], ig[:])
            else:
                fc = spool.tile([128, KCH, BL], F32, tag=f"fc{l}")
                nc.vector.tensor_mul(fc[:], sig_fi[:, 0:4, :], c_prev[:])
                nc.vector.tensor_mul(ig[:], sig_fi[:, 4:8, :], tanh_g[:])
                nc.vector.tensor_add(c_new[:], fc[:], ig[:])
            tanh_c = spool.tile([128, KCH, BL], F32, tag=f"tc{l}")
            nc.scalar.activation(tanh_c[:], c_new[:], AF.Tanh)
            h_new = hpool.tile([128, KCH, BL], BF16, tag=f"h{l}")
            nc.vector.tensor_mul(h_new[:], sig_o[:], tanh_c[:])
            return c_new, h_new

        def alloc_g1(step):
            ga1 = gpool.tile([128, 8, BL], F32, tag="ga1")
            gb1 = gpool.tile([128, 8, BL], F32, tag="gb1")
            emit_bias1(ga1, gb1, only=False)
            return ga1, gb1

        n_steps = n_tf + n_ar
        h0 = h1 = c0 = c1 = None
        dout = None
        ga0 = gb0 = ga1 = gb1 = None
        for t in range(n_steps):
            if t == 0:
                ga0 = gpool.tile([128, 8, BL], F32, tag="ga0")
                gb0 = gpool.tile([128, 8, BL], F32, tag="gb0")
                emit_xa(ga0, gb0, x_sb[:, 0, :], only=True)
            c0, h0 = cell(ga0, gb0, c0, 0)
            if t == 0:
                ga1, gb1 = alloc_g1(0)
            else:
                emit_rec(whh1_sb, h1, ga1, gb1, last=False)
            emit_rec(wih1_sb, h0, ga1, gb1, last=True)
            c1, h1 = cell(ga1, gb1, c1, 1)

            # In AR steps, issue the next step's bias matmuls BEFORE the
            # decoder: they only need the already-consumed layer-1 gate
            # PSUM, so they fill part of PE's wait for h1. The decoder
            # PSUM aliases gb0's slot there (its bank-zeroing start flag
            # is neutralized by xa's own m%4==0 start pattern next step).
            next_is_ar = n_tf <= t + 1 < n_steps
            if next_is_ar:
                ga1n, gb1n = alloc_g1(t + 1)

            if t >= n_tf - 1:
                j = t - (n_tf - 1)
                dec_ps = gpool.tile([FD, BL], F32, tag="gb0")
                for k in range(KCH):
                    nc.tensor.matmul(dec_ps[:], wdec_sb[:, k, :], h1[:, k, :],
                                     start=(k == 0), stop=(k == KCH - 1))
                dout = dpool.tile([FD, BL], BF16, tag="dout")
                nc.scalar.add(dout[:], dec_ps[:], bdec_sb[:])
                nc.sync.dma_start(y_d[j], dout[0:F, :])

            if t + 1 < n_steps:
                ga0 = gpool.tile([128, 8, BL], F32, tag="ga0")
                gb0 = gpool.tile([128, 8, BL], F32, tag="gb0")
                rhs = x_sb[:, t + 1, :] if t + 1 < n_tf else dout[0:KX, :]
                emit_xa(ga0, gb0, rhs, only=False)
                emit_rec(whh0_sb, h0, ga0, gb0, last=True)
                ga1, gb1 = (ga1n, gb1n) if next_is_ar else alloc_g1(t + 1)

    nc.compile()
    return nc


def _get_program(n_tf=S, n_ar=PRED_LEN - 1):
    key = (n_tf, n_ar)
    with _cache_lock:
        if key not in _cache:
            _cache[key] = _build_program(n_tf, n_ar)
        return _cache[key]


def _kmajor(w):
    """[H, N] -> [128, KCH, N]: row h = k*128 + p lands at [p, k, :]."""
    n = w.shape[1]
    return np.ascontiguousarray(
        w.reshape(KCH, 128, n).transpose(1, 0, 2)).astype(BF16NP)


def _prep_weights(W_enc, b_enc, Wih0, Whh0, bih0, bhh0,
                  Wih1, Whh1, bih1, bhh1, W_dec, b_dec):
    perm = _gate_perm()
    f32 = np.float32

    M0 = (Wih0 @ W_enc)[perm]                                   # [G, F]
    b0 = (Wih0 @ b_enc + bih0 + bhh0)[perm]                     # [G]
    m0t = np.concatenate([M0.T, b0[None, :]], axis=0)           # [KX, G]

    b1p = (bih1 + bhh1)[perm]                                   # [G]
    wdec_aug = np.concatenate(
        [W_dec.T, np.zeros((H, 2), f32)], axis=1)               # [H, FD]
    bdec = np.concatenate([b_dec, np.ones((1,), f32), np.zeros((1,), f32)])

    return {
        "m0t": np.ascontiguousarray(m0t).astype(BF16NP),
        "whh0t": _kmajor(np.ascontiguousarray(Whh0[perm].T)),
        "wih1t": _kmajor(np.ascontiguousarray(Wih1[perm].T)),
        "whh1t": _kmajor(np.ascontiguousarray(Whh1[perm].T)),
        "b1": b1p[None, :].astype(BF16NP),
        "ones": np.ones((1, BL), BF16NP),
        "wdect": _kmajor(wdec_aug),
        "bdec": np.ascontiguousarray(bdec[:, None], f32),
    }


def _make_in_maps(x, weights, _n_tf=S):
    in_maps = []
    for c in range(NCORES):
        xs = x[c * BL:(c + 1) * BL, :_n_tf, :]                # [BL, n_tf, F]
        xT = xs.transpose(2, 1, 0)                            # [F, n_tf, BL]
        xa = np.concatenate(
            [xT, np.ones((1, _n_tf, BL), np.float32)], axis=0)  # [KX, n_tf, BL]
        in_maps.append(
            {"xT": np.ascontiguousarray(xa).astype(BF16NP), **weights})
    return in_maps


def kernel(x, W_enc, b_enc, Wih0, Whh0, bih0, bhh0,
           Wih1, Whh1, bih1, bhh1, W_dec, b_dec, _n_tf=S, _n_ar=PRED_LEN - 1):
    from concourse.bass_utils import run_bass_kernel_spmd

    x = np.asarray(x, np.float32)
    weights = _prep_weights(
        np.asarray(W_enc, np.float32), np.asarray(b_enc, np.float32),
        np.asarray(Wih0, np.float32), np.asarray(Whh0, np.float32),
        np.asarray(bih0, np.float32), np.asarray(bhh0, np.float32),
        np.asarray(Wih1, np.float32), np.asarray(Whh1, np.float32),
        np.asarray(bih1, np.float32), np.asarray(bhh1, np.float32),
        np.asarray(W_dec, np.float32), np.asarray(b_dec, np.float32))

    nc = _get_program(_n_tf, _n_ar)
    in_maps = _make_in_maps(x, weights, _n_tf)
    res = run_bass_kernel_spmd(nc, in_maps, core_ids=list(range(NCORES)))

    out = np.empty((B, _n_ar + 1, F), np.float32)
    for c in range(NCORES):
        y = np.asarray(res.results[c]["y"], dtype=np.float32)  # [n_out, F, BL]
        out[c * BL:(c + 1) * BL] = y.transpose(2, 0, 1)
    return out


# revision 22
# speedup vs baseline: 1.3326x; 1.0079x over previous
"""Trainium2 Bass kernel for a 2-layer LSTM encoder/decoder forecaster.

Model (per batch element):
  teacher-forced over S=168 steps:  enc -> LSTM0 -> LSTM1 (keep last out)
  autoregressive rollout for 23 more steps feeding decoder output back.

Sharding: data-parallel, batch 1024 -> 8 cores x 128. All weights are
replicated and SBUF-resident; zero inter-core communication.

Layout: everything is FEATURE-MAJOR. Gates are computed as 16 chunks of
[128 gate-rows x 128 batch] PSUM tiles, with the (transposed, chunked)
weights as the stationary matmul operand and h / x as the moving
operand. Because the cell elementwise output h = sig(o)*tanh(c) is then
produced directly in [feature, batch] layout, it is ALREADY the k-chunk
operand the next step's recurrent matmuls need - no PE transposes, no
PSUM->SBUF copies of state anywhere in the loop.

Everything is bf16 on the matmul path (full PE rate at any width, and a
validated ~3e-3 end-to-end error vs the 2e-2 budget; fp8 was measured at
4.3e-2 and rejected). The cell state c stays fp32.

Gate chunk order after host-side row permutation: (f, i, o, g) so that
GA = [f|i] (chunks 0..7) and GB = [o|g] (chunks 8..15), letting each
activation read one contiguous PSUM span.

The encoder is algebraically fused into layer 0 (M0 = Wih0 @ W_enc, bias
folded into an appended ones-row of the feature-major input). Layer 1's
bias enters via 16 K=1 matmuls against a resident ones vector. The
decoder is augmented with a column that regenerates the ones-row so the
AR feedback tile needs no fixup at all: the decoder's SBUF output IS the
next step's input operand.

PE order per steady-state step: [xa(t), whh0(t), bias1(t)] prefetched at
the end of step t-1, then whh1(t), wih1(t) - chosen so every matmul's
input is ready before PE reaches it, keeping PE (the bottleneck at ~28.7k
cycles/step) gapless and at full p-state.
"""

import sys
import threading

sys.path.insert(0, "/opt/trn_rl_repo")

import numpy as np
import ml_dtypes

PRED_LEN = 24
F, I, H = 64, 128, 512
B, S = 1024, 168
NCORES = 8
BL = B // NCORES          # batch per core = 128
G = 4 * H                 # gate width 2048
NCH = G // 128            # 16 gate chunks
KCH = H // 128            # 4 k-chunks of the hidden dim
KX = F + 1                # x operand rows incl. ones row = 65
FD = F + 2                # decoder rows: 64 outputs + ones + pad = 66

BF16NP = ml_dtypes.bfloat16

_cache = {}
_cache_lock = threading.Lock()


def _gate_perm():
    # pytorch gate order i,f,g,o -> reorder rows to (f,i,o,g): chunks
    # 0-3=f, 4-7=i (-> GA), 8-11=o, 12-15=g (-> GB).
    return np.concatenate([
        np.arange(H, 2 * H),        # f
        np.arange(0, H),            # i
        np.arange(3 * H, 4 * H),    # o
        np.arange(2 * H, 3 * H),    # g
    ])


def _build_program(n_tf=S, n_ar=PRED_LEN - 1):
    import concourse.bacc as bacc
    import concourse.tile as tile
    import concourse.mybir as mybir

    F32 = mybir.dt.float32
    BF16 = mybir.dt.bfloat16
    AF = mybir.ActivationFunctionType

    nc = bacc.Bacc("TRN2", target_bir_lowering=False, debug=False,
                   num_devices=NCORES)

    x_d = nc.dram_tensor("xT", [KX, n_tf, BL], BF16, kind="ExternalInput").ap()
    m0_d = nc.dram_tensor("m0t", [KX, G], BF16, kind="ExternalInput").ap()
    whh0_d = nc.dram_tensor("whh0t", [128, KCH, G], BF16, kind="ExternalInput").ap()
    wih1_d = nc.dram_tensor("wih1t", [128, KCH, G], BF16, kind="ExternalInput").ap()
    whh1_d = nc.dram_tensor("whh1t", [128, KCH, G], BF16, kind="ExternalInput").ap()
    b1_d = nc.dram_tensor("b1", [1, G], BF16, kind="ExternalInput").ap()
    ones_d = nc.dram_tensor("ones", [1, BL], BF16, kind="ExternalInput").ap()
    wdec_d = nc.dram_tensor("wdect", [128, KCH, FD], BF16, kind="ExternalInput").ap()
    bdec_d = nc.dram_tensor("bdec", [FD, 1], F32, kind="ExternalInput").ap()
    y_d = nc.dram_tensor("y", [n_ar + 1, F, BL], BF16, kind="ExternalOutput").ap()

    from contextlib import ExitStack
    with tile.TileContext(nc) as tc, ExitStack() as ctx:
        wpool = ctx.enter_context(tc.tile_pool(name="w", bufs=1))
        spool = ctx.enter_context(tc.tile_pool(name="s", bufs=2))
        hpool = ctx.enter_context(tc.tile_pool(name="h", bufs=2))
        dpool = ctx.enter_context(tc.tile_pool(name="d", bufs=2))
        gpool = ctx.enter_context(tc.tile_pool(name="g", bufs=1, space="PSUM"))

        # ---- resident weights + input ----
        # DMA order matters: everything step 0 touches (x head, m0, b1,
        # ones) loads FIRST; the three 2MB weight matrices follow in
        # first-use order; the 2.7MB x tail (not needed until step 8)
        # goes last. This cut a measured ~25us lead-in stall where
        # bias1(0)/wih1(0) sat behind the bulk transfers.
        x_sb = wpool.tile([KX, n_tf, BL], BF16)
        x_head = min(8, n_tf)
        nc.sync.dma_start(x_sb[:, 0:x_head, :], x_d[:, 0:x_head, :])
        m0_sb = wpool.tile([KX, G], BF16)
        nc.sync.dma_start(m0_sb[:], m0_d[:])
        b1_sb = wpool.tile([1, G], BF16)
        nc.sync.dma_start(b1_sb[:], b1_d[:])
        ones_sb = wpool.tile([1, BL], BF16)
        nc.sync.dma_start(ones_sb[:], ones_d[:])
        wih1_sb = wpool.tile([128, KCH, G], BF16)
        nc.sync.dma_start(wih1_sb[:], wih1_d[:])
        whh0_sb = wpool.tile([128, KCH, G], BF16)
        nc.sync.dma_start(whh0_sb[:], whh0_d[:])
        whh1_sb = wpool.tile([128, KCH, G], BF16)
        nc.sync.dma_start(whh1_sb[:], whh1_d[:])
        wdec_sb = wpool.tile([128, KCH, FD], BF16)
        nc.sync.dma_start(wdec_sb[:], wdec_d[:])
        bdec_sb = wpool.tile([FD, 1], F32)
        nc.sync.dma_start(bdec_sb[:], bdec_d[:])
        if x_head < n_tf:
            nc.sync.dma_start(x_sb[:, x_head:, :], x_d[:, x_head:, :])

        def halves(ga, gb, m):
            return (ga if m < 8 else gb)[:, m % 8, :]

        # start=True lazily zeroes the WHOLE 2KB PSUM bank (4 of our 512B
        # chunk regions), so only the first write per bank may set it; the
        # sibling regions still see the bank's pending-zero and overwrite.
        def emit_xa(ga, gb, rhs, only):
            # input-side gate contribution; `only`=True closes the group
            # (t=0 has no recurrent term).
            for m in range(NCH):
                nc.tensor.matmul(halves(ga, gb, m),
                                 m0_sb[:, m * 128:(m + 1) * 128], rhs,
                                 start=(m % 4 == 0), stop=only,
                                 skip_group_check=True)

        def emit_bias1(ga, gb, only):
            for m in range(NCH):
                nc.tensor.matmul(halves(ga, gb, m),
                                 b1_sb[:, m * 128:(m + 1) * 128], ones_sb[:],
                                 start=(m % 4 == 0), stop=only,
                                 skip_group_check=True)

        def emit_rec(w_sb, h, ga, gb, last):
            for m in range(NCH):
                out = halves(ga, gb, m)
                for k in range(KCH):
                    nc.tensor.matmul(out,
                                     w_sb[:, k, m * 128:(m + 1) * 128],
                                     h[:, k, :],
                                     start=False, stop=(last and k == KCH - 1),
                                     skip_group_check=True)

        def cell(ga, gb, c_prev, l):
            sig_fi = spool.tile([128, 8, BL], F32, tag=f"sfi{l}")
            nc.scalar.activation(sig_fi[:], ga[:], AF.Sigmoid)
            tanh_g = spool.tile([128, KCH, BL], F32, tag=f"tg{l}")
            nc.scalar.activation(tanh_g[:], gb[:, 4:8, :], AF.Tanh)
            sig_o = spool.tile([128, KCH, BL], F32, tag=f"so{l}")
            nc.scalar.activation(sig_o[:], gb[:, 0:4, :], AF.Sigmoid)
            # fc first on the DVE queue: its input sig_fi is ready one Act
            # op earlier than ig's tanh_g, shortening the serial cell chain.
            c_new = hpool.tile([128, KCH, BL], F32, tag=f"c{l}")
            ig = spool.tile([128, KCH, BL], F32, tag=f"ig{l}")
            if c_prev is None:
                nc.vector.tensor_mul(ig[:], sig_fi[:, 4:8, :], tanh_g[:])
                nc.vector.tensor_copy(c_new[:], ig[:])
            else:
                fc = spool.tile([128, KCH, BL], F32, tag=f"fc{l}")
                nc.vector.tensor_mul(fc[:], sig_fi[:, 0:4, :], c_prev[:])
                nc.vector.tensor_mul(ig[:], sig_fi[:, 4:8, :], tanh_g[:])
                nc.vector.tensor_add(c_new[:], fc[:], ig[:])
            tanh_c = spool.tile([128, KCH, BL], F32, tag=f"tc{l}")
            nc.scalar.activation(tanh_c[:], c_new[:], AF.Tanh)
            h_new = hpool.tile([128, KCH, BL], BF16, tag=f"h{l}")
            nc.vector.tensor_mul(h_new[:], sig_o[:], tanh_c[:])
            return c_new, h_new

        def alloc_g1(step):
            ga1 = gpool.tile([128, 8, BL], F32, tag="ga1")
            gb1 = gpool.tile([128, 8, BL], F32, tag="gb1")
            emit_bias1(ga1, gb1, only=False)
            return ga1, gb1

        n_steps = n_tf + n_ar
        h0 = h1 = c0 = c1 = None
        dout = None
        ga0 = gb0 = ga1 = gb1 = None
        for t in range(n_steps):
            if t == 0:
                ga0 = gpool.tile([128, 8, BL], F32, tag="ga0")
                gb0 = gpool.tile([128, 8, BL], F32, tag="gb0")
                emit_xa(ga0, gb0, x_sb[:, 0, :], only=True)
            c0, h0 = cell(ga0, gb0, c0, 0)
            if t == 0:
                ga1, gb1 = alloc_g1(0)
            else:
                emit_rec(whh1_sb, h1, ga1, gb1, last=False)
            emit_rec(wih1_sb, h0, ga1, gb1, last=True)
            c1, h1 = cell(ga1, gb1, c1, 1)

            # In AR steps, issue the next step's bias matmuls BEFORE the
            # decoder: they only need the already-consumed layer-1 gate
            # PSUM, so they fill part of PE's wait for h1. The decoder
            # PSUM aliases gb0's slot there (its bank-zeroing start flag
            # is neutralized by xa's own m%4==0 start pattern next step).
            next_is_ar = n_tf <= t + 1 < n_steps
            if next_is_ar:
                ga1n, gb1n = alloc_g1(t + 1)

            if t >= n_tf - 1:
                j = t - (n_tf - 1)
                dec_ps = gpool.tile([FD, BL], F32, tag="gb0")
                for k in range(KCH):
                    nc.tensor.matmul(dec_ps[:], wdec_sb[:, k, :], h1[:, k, :],
                                     start=(k == 0), stop=(k == KCH - 1))
                dout = dpool.tile([FD, BL], BF16, tag="dout")
                nc.scalar.add(dout[:], dec_ps[:], bdec_sb[:])
                nc.sync.dma_start(y_d[j], dout[0:F, :])

            if t + 1 < n_steps:
                ga0 = gpool.tile([128, 8, BL], F32, tag="ga0")
                gb0 = gpool.tile([128, 8, BL], F32, tag="gb0")
                rhs = x_sb[:, t + 1, :] if t + 1 < n_tf else dout[0:KX, :]
                emit_xa(ga0, gb0, rhs, only=False)
                emit_rec(whh0_sb, h0, ga0, gb0, last=True)
                ga1, gb1 = (ga1n, gb1n) if next_is_ar else alloc_g1(t + 1)

    nc.compile()
    return nc


def _get_program(n_tf=S, n_ar=PRED_LEN - 1):
    key = (n_tf, n_ar)
    with _cache_lock:
        if key not in _cache:
            _cache[key] = _build_program(n_tf, n_ar)
        return _cache[key]


def _kmajor(w):
    """[H, N] -> [128, KCH, N]: row h = k*128 + p lands at [p, k, :]."""
    n = w.shape[1]
    return np.ascontiguousarray(
        w.reshape(KCH, 128, n).transpose(1, 0, 2)).astype(BF16NP)


def _prep_weights(W_enc, b_enc, Wih0, Whh0, bih0, bhh0,
                  Wih1, Whh1, bih1, bhh1, W_dec, b_dec):
    perm = _gate_perm()
    f32 = np.float32

    M0 = (Wih0 @ W_enc)[perm]                                   # [G, F]
    b0 = (Wih0 @ b_enc + bih0 + bhh0)[perm]                     # [G]
    m0t = np.concatenate([M0.T, b0[None, :]], axis=0)           # [KX, G]

    b1p = (bih1 + bhh1)[perm]                                   # [G]
    wdec_aug = np.concatenate(
        [W_dec.T, np.zeros((H, 2), f32)], axis=1)               # [H, FD]
    bdec = np.concatenate([b_dec, np.ones((1,), f32), np.zeros((1,), f32)])

    return {
        "m0t": np.ascontiguousarray(m0t).astype(BF16NP),
        "whh0t": _kmajor(np.ascontiguousarray(Whh0[perm].T)),
        "wih1t": _kmajor(np.ascontiguousarray(Wih1[perm].T)),
        "whh1t": _kmajor(np.ascontiguousarray(Whh1[perm].T)),
        "b1": b1p[None, :].astype(BF16NP),
        "ones": np.ones((1, BL), BF16NP),
        "wdect": _kmajor(wdec_aug),
        "bdec": np.ascontiguousarray(bdec[:, None], f32),
    }


def _make_in_maps(x, weights, _n_tf=S):
    in_maps = []
    for c in range(NCORES):
        xs = x[c * BL:(c + 1) * BL, :_n_tf, :]                # [BL, n_tf, F]
        xT = xs.transpose(2, 1, 0)                            # [F, n_tf, BL]
        xa = np.concatenate(
            [xT, np.ones((1, _n_tf, BL), np.float32)], axis=0)  # [KX, n_tf, BL]
        in_maps.append(
            {"xT": np.ascontiguousarray(xa).astype(BF16NP), **weights})
    return in_maps


def kernel(x, W_enc, b_enc, Wih0, Whh0, bih0, bhh0,
           Wih1, Whh1, bih1, bhh1, W_dec, b_dec, _n_tf=S, _n_ar=PRED_LEN - 1):
    from concourse.bass_utils import run_bass_kernel_spmd

    x = np.asarray(x, np.float32)
    weights = _prep_weights(
        np.asarray(W_enc, np.float32), np.asarray(b_enc, np.float32),
        np.asarray(Wih0, np.float32), np.asarray(Whh0, np.float32),
        np.asarray(bih0, np.float32), np.asarray(bhh0, np.float32),
        np.asarray(Wih1, np.float32), np.asarray(Whh1, np.float32),
        np.asarray(bih1, np.float32), np.asarray(bhh1, np.float32),
        np.asarray(W_dec, np.float32), np.asarray(b_dec, np.float32))

    nc = _get_program(_n_tf, _n_ar)
    in_maps = _make_in_maps(x, weights, _n_tf)
    res = run_bass_kernel_spmd(nc, in_maps, core_ids=list(range(NCORES)))

    out = np.empty((B, _n_ar + 1, F), np.float32)
    for c in range(NCORES):
        y = np.asarray(res.results[c]["y"], dtype=np.float32)  # [n_out, F, BL]
        out[c * BL:(c + 1) * BL] = y.transpose(2, 0, 1)
    return out


# revision 24
# speedup vs baseline: 1.3378x; 1.0039x over previous
"""Trainium2 Bass kernel for a 2-layer LSTM encoder/decoder forecaster.

Model (per batch element):
  teacher-forced over S=168 steps:  enc -> LSTM0 -> LSTM1 (keep last out)
  autoregressive rollout for 23 more steps feeding decoder output back.

Sharding: data-parallel, batch 1024 -> 8 cores x 128. All weights are
replicated and SBUF-resident; zero inter-core communication.

Layout: everything is FEATURE-MAJOR. Gates are computed as 16 chunks of
[128 gate-rows x 128 batch] PSUM tiles, with the (transposed, chunked)
weights as the stationary matmul operand and h / x as the moving
operand. Because the cell elementwise output h = sig(o)*tanh(c) is then
produced directly in [feature, batch] layout, it is ALREADY the k-chunk
operand the next step's recurrent matmuls need - no PE transposes, no
PSUM->SBUF copies of state anywhere in the loop.

Everything is bf16 on the matmul path (full PE rate at any width, and a
validated ~3e-3 end-to-end error vs the 2e-2 budget; fp8 was measured at
4.3e-2 and rejected). The cell state c stays fp32.

Gate chunk order after host-side row permutation: (f, i, o, g) so that
GA = [f|i] (chunks 0..7) and GB = [o|g] (chunks 8..15), letting each
activation read one contiguous PSUM span.

The encoder is algebraically fused into layer 0 (M0 = Wih0 @ W_enc, bias
folded into an appended ones-row of the feature-major input). Layer 1's
bias enters via 16 K=1 matmuls against a resident ones vector. The
decoder is augmented with a column that regenerates the ones-row so the
AR feedback tile needs no fixup at all: the decoder's SBUF output IS the
next step's input operand.

PE order per steady-state step: [xa(t), whh0(t), bias1(t)] prefetched at
the end of step t-1, then whh1(t), wih1(t) - chosen so every matmul's
input is ready before PE reaches it, keeping PE (the bottleneck at ~28.7k
cycles/step) gapless and at full p-state.
"""

import sys
import threading

sys.path.insert(0, "/opt/trn_rl_repo")

import numpy as np
import ml_dtypes

PRED_LEN = 24
F, I, H = 64, 128, 512
B, S = 1024, 168
NCORES = 8
BL = B // NCORES          # batch per core = 128
G = 4 * H                 # gate width 2048
NCH = G // 128            # 16 gate chunks
KCH = H // 128            # 4 k-chunks of the hidden dim
KX = F + 1                # x operand rows incl. ones row = 65
FD = F + 2                # decoder rows: 64 outputs + ones + pad = 66

BF16NP = ml_dtypes.bfloat16

_cache = {}
_cache_lock = threading.Lock()


def _gate_perm():
    # pytorch gate order i,f,g,o -> reorder rows to (f,i,o,g): chunks
    # 0-3=f, 4-7=i (-> GA), 8-11=o, 12-15=g (-> GB).
    return np.concatenate([
        np.arange(H, 2 * H),        # f
        np.arange(0, H),            # i
        np.arange(3 * H, 4 * H),    # o
        np.arange(2 * H, 3 * H),    # g
    ])


def _build_program(n_tf=S, n_ar=PRED_LEN - 1):
    import concourse.bacc as bacc
    import concourse.tile as tile
    import concourse.mybir as mybir

    F32 = mybir.dt.float32
    BF16 = mybir.dt.bfloat16
    AF = mybir.ActivationFunctionType

    nc = bacc.Bacc("TRN2", target_bir_lowering=False, debug=False,
                   num_devices=NCORES)

    x_d = nc.dram_tensor("xT", [KX, n_tf, BL], BF16, kind="ExternalInput").ap()
    m0_d = nc.dram_tensor("m0t", [KX, G], BF16, kind="ExternalInput").ap()
    whh0_d = nc.dram_tensor("whh0t", [128, KCH, G], BF16, kind="ExternalInput").ap()
    wih1_d = nc.dram_tensor("wih1t", [128, KCH, G], BF16, kind="ExternalInput").ap()
    whh1_d = nc.dram_tensor("whh1t", [128, KCH, G], BF16, kind="ExternalInput").ap()
    b1t_d = nc.dram_tensor("b1t", [KCH, KCH, 128], BF16, kind="ExternalInput").ap()
    sel4_d = nc.dram_tensor("sel4", [KCH, 512], BF16, kind="ExternalInput").ap()
    wdec_d = nc.dram_tensor("wdect", [128, KCH, FD], BF16, kind="ExternalInput").ap()
    bdec_d = nc.dram_tensor("bdec", [FD, 1], F32, kind="ExternalInput").ap()
    y_d = nc.dram_tensor("y", [n_ar + 1, F, BL], BF16, kind="ExternalOutput").ap()

    from contextlib import ExitStack
    with tile.TileContext(nc) as tc, ExitStack() as ctx:
        wpool = ctx.enter_context(tc.tile_pool(name="w", bufs=1))
        spool = ctx.enter_context(tc.tile_pool(name="s", bufs=2))
        hpool = ctx.enter_context(tc.tile_pool(name="h", bufs=2))
        dpool = ctx.enter_context(tc.tile_pool(name="d", bufs=2))
        gpool = ctx.enter_context(tc.tile_pool(name="g", bufs=1, space="PSUM"))

        # ---- resident weights + input ----
        # DMA order matters: everything step 0 touches (x head, m0, b1,
        # ones) loads FIRST; the three 2MB weight matrices follow in
        # first-use order; the 2.7MB x tail (not needed until step 8)
        # goes last. This cut a measured ~25us lead-in stall where
        # bias1(0)/wih1(0) sat behind the bulk transfers.
        x_sb = wpool.tile([KX, n_tf, BL], BF16)
        x_head = min(8, n_tf)
        nc.sync.dma_start(x_sb[:, 0:x_head, :], x_d[:, 0:x_head, :])
        m0_sb = wpool.tile([KX, G], BF16)
        nc.sync.dma_start(m0_sb[:], m0_d[:])
        b1t_sb = wpool.tile([KCH, KCH, 128], BF16)
        nc.sync.dma_start(b1t_sb[:], b1t_d[:])
        sel4_sb = wpool.tile([KCH, 512], BF16)
        nc.sync.dma_start(sel4_sb[:], sel4_d[:])
        wih1_sb = wpool.tile([128, KCH, G], BF16)
        nc.sync.dma_start(wih1_sb[:], wih1_d[:])
        whh0_sb = wpool.tile([128, KCH, G], BF16)
        nc.sync.dma_start(whh0_sb[:], whh0_d[:])
        whh1_sb = wpool.tile([128, KCH, G], BF16)
        nc.sync.dma_start(whh1_sb[:], whh1_d[:])
        wdec_sb = wpool.tile([128, KCH, FD], BF16)
        nc.sync.dma_start(wdec_sb[:], wdec_d[:])
        bdec_sb = wpool.tile([FD, 1], F32)
        nc.sync.dma_start(bdec_sb[:], bdec_d[:])
        if x_head < n_tf:
            nc.sync.dma_start(x_sb[:, x_head:, :], x_d[:, x_head:, :])

        def halves(ga, gb, m):
            return (ga if m < 8 else gb)[:, m % 8, :]

        # start=True lazily zeroes the WHOLE 2KB PSUM bank (4 of our 512B
        # chunk regions), so only the first write per bank may set it; the
        # sibling regions still see the bank's pending-zero and overwrite.
        def emit_xa(ga, gb, rhs, only):
            # input-side gate contribution; `only`=True closes the group
            # (t=0 has no recurrent term).
            for m in range(NCH):
                nc.tensor.matmul(halves(ga, gb, m),
                                 m0_sb[:, m * 128:(m + 1) * 128], rhs,
                                 start=(m % 4 == 0), stop=only,
                                 skip_group_check=True)

        def emit_bias1(ga, gb, only):
            # One N=512 matmul per PSUM bank: out[p,c,b] = sum_k
            # b1t[k,j,p] * sel4[k, c*128+b] = b1[(4j+c)*128+p] - the
            # per-(partition, chunk) bias broadcast. Same stream cycles
            # as 16 K=1 matmuls but 12 fewer instruction pairs, and each
            # bank's start=True covers every byte it zeroes.
            for j in range(KCH):
                gt = ga if j < 2 else gb
                out = gt[:, (j % 2) * 4:(j % 2) * 4 + 4, :]
                nc.tensor.matmul(out, b1t_sb[:, j, :], sel4_sb[:],
                                 start=True, stop=only,
                                 skip_group_check=True)

        def emit_rec(w_sb, h, ga, gb, last):
            for m in range(NCH):
                out = halves(ga, gb, m)
                for k in range(KCH):
                    nc.tensor.matmul(out,
                                     w_sb[:, k, m * 128:(m + 1) * 128],
                                     h[:, k, :],
                                     start=False, stop=(last and k == KCH - 1),
                                     skip_group_check=True)

        def cell(ga, gb, c_prev, l):
            sig_fi = spool.tile([128, 8, BL], F32, tag=f"sfi{l}")
            nc.scalar.activation(sig_fi[:], ga[:], AF.Sigmoid)
            tanh_g = spool.tile([128, KCH, BL], F32, tag=f"tg{l}")
            nc.scalar.activation(tanh_g[:], gb[:, 4:8, :], AF.Tanh)
            sig_o = spool.tile([128, KCH, BL], F32, tag=f"so{l}")
            nc.scalar.activation(sig_o[:], gb[:, 0:4, :], AF.Sigmoid)
            # fc first on the DVE queue: its input sig_fi is ready one Act
            # op earlier than ig's tanh_g, shortening the serial cell chain.
            c_new = hpool.tile([128, KCH, BL], F32, tag=f"c{l}")
            ig = spool.tile([128, KCH, BL], F32, tag=f"ig{l}")
            if c_prev is None:
                nc.vector.tensor_mul(ig[:], sig_fi[:, 4:8, :], tanh_g[:])
                nc.vector.tensor_copy(c_new[:], ig[:])
            else:
                fc = spool.tile([128, KCH, BL], F32, tag=f"fc{l}")
                nc.vector.tensor_mul(fc[:], sig_fi[:, 0:4, :], c_prev[:])
                nc.vector.tensor_mul(ig[:], sig_fi[:, 4:8, :], tanh_g[:])
                nc.vector.tensor_add(c_new[:], fc[:], ig[:])
            tanh_c = spool.tile([128, KCH, BL], F32, tag=f"tc{l}")
            nc.scalar.activation(tanh_c[:], c_new[:], AF.Tanh)
            h_new = hpool.tile([128, KCH, BL], BF16, tag=f"h{l}")
            for k in range(KCH):
                nc.vector.tensor_mul(h_new[:, k, :], sig_o[:, k, :],
                                     tanh_c[:, k, :])
            return c_new, h_new

        def alloc_g1(step):
            ga1 = gpool.tile([128, 8, BL], F32, tag="ga1")
            gb1 = gpool.tile([128, 8, BL], F32, tag="gb1")
            emit_bias1(ga1, gb1, only=False)
            return ga1, gb1

        n_steps = n_tf + n_ar
        h0 = h1 = c0 = c1 = None
        dout = None
        ga0 = gb0 = ga1 = gb1 = None
        for t in range(n_steps):
            if t == 0:
                ga0 = gpool.tile([128, 8, BL], F32, tag="ga0")
                gb0 = gpool.tile([128, 8, BL], F32, tag="gb0")
                emit_xa(ga0, gb0, x_sb[:, 0, :], only=True)
            c0, h0 = cell(ga0, gb0, c0, 0)
            if t == 0:
                ga1, gb1 = alloc_g1(0)
            else:
                emit_rec(whh1_sb, h1, ga1, gb1, last=False)
            emit_rec(wih1_sb, h0, ga1, gb1, last=True)
            c1, h1 = cell(ga1, gb1, c1, 1)

            # In AR steps, issue the next step's bias matmuls BEFORE the
            # decoder: they only need the already-consumed layer-1 gate
            # PSUM, so they fill part of PE's wait for h1. The decoder
            # PSUM aliases gb0's slot there (its bank-zeroing start flag
            # is neutralized by xa's own m%4==0 start pattern next step).
            next_is_ar = n_tf <= t + 1 < n_steps
            if next_is_ar:
                ga1n, gb1n = alloc_g1(t + 1)

            if t >= n_tf - 1:
                j = t - (n_tf - 1)
                dec_ps = gpool.tile([FD, BL], F32, tag="gb0")
                for k in range(KCH):
                    nc.tensor.matmul(dec_ps[:], wdec_sb[:, k, :], h1[:, k, :],
                                     start=(k == 0), stop=(k == KCH - 1))
                dout = dpool.tile([FD, BL], BF16, tag="dout")
                nc.scalar.add(dout[:], dec_ps[:], bdec_sb[:])
                nc.sync.dma_start(y_d[j], dout[0:F, :])

            if t + 1 < n_steps:
                ga0 = gpool.tile([128, 8, BL], F32, tag="ga0")
                gb0 = gpool.tile([128, 8, BL], F32, tag="gb0")
                rhs = x_sb[:, t + 1, :] if t + 1 < n_tf else dout[0:KX, :]
                emit_xa(ga0, gb0, rhs, only=False)
                emit_rec(whh0_sb, h0, ga0, gb0, last=True)
                ga1, gb1 = (ga1n, gb1n) if next_is_ar else alloc_g1(t + 1)

    nc.compile()
    return nc


def _get_program(n_tf=S, n_ar=PRED_LEN - 1):
    key = (n_tf, n_ar)
    with _cache_lock:
        if key not in _cache:
            _cache[key] = _build_program(n_tf, n_ar)
        return _cache[key]


def _kmajor(w):
    """[H, N] -> [128, KCH, N]: row h = k*128 + p lands at [p, k, :]."""
    n = w.shape[1]
    return np.ascontiguousarray(
        w.reshape(KCH, 128, n).transpose(1, 0, 2)).astype(BF16NP)


def _prep_weights(W_enc, b_enc, Wih0, Whh0, bih0, bhh0,
                  Wih1, Whh1, bih1, bhh1, W_dec, b_dec):
    perm = _gate_perm()
    f32 = np.float32

    M0 = (Wih0 @ W_enc)[perm]                                   # [G, F]
    b0 = (Wih0 @ b_enc + bih0 + bhh0)[perm]                     # [G]
    m0t = np.concatenate([M0.T, b0[None, :]], axis=0)           # [KX, G]

    b1p = (bih1 + bhh1)[perm]                                   # [G]
    wdec_aug = np.concatenate(
        [W_dec.T, np.zeros((H, 2), f32)], axis=1)               # [H, FD]
    bdec = np.concatenate([b_dec, np.ones((1,), f32), np.zeros((1,), f32)])

    return {
        "m0t": np.ascontiguousarray(m0t).astype(BF16NP),
        "whh0t": _kmajor(np.ascontiguousarray(Whh0[perm].T)),
        "wih1t": _kmajor(np.ascontiguousarray(Wih1[perm].T)),
        "whh1t": _kmajor(np.ascontiguousarray(Whh1[perm].T)),
        "b1t": np.ascontiguousarray(
            b1p.reshape(KCH, KCH, 128).transpose(1, 0, 2)).astype(BF16NP),
        "sel4": np.kron(np.eye(KCH, dtype=f32),
                        np.ones((1, 128), f32)).astype(BF16NP),
        "wdect": _kmajor(wdec_aug),
        "bdec": np.ascontiguousarray(bdec[:, None], f32),
    }


def _make_in_maps(x, weights, _n_tf=S):
    in_maps = []
    for c in range(NCORES):
        xs = x[c * BL:(c + 1) * BL, :_n_tf, :]                # [BL, n_tf, F]
        xT = xs.transpose(2, 1, 0)                            # [F, n_tf, BL]
        xa = np.concatenate(
            [xT, np.ones((1, _n_tf, BL), np.float32)], axis=0)  # [KX, n_tf, BL]
        in_maps.append(
            {"xT": np.ascontiguousarray(xa).astype(BF16NP), **weights})
    return in_maps


def kernel(x, W_enc, b_enc, Wih0, Whh0, bih0, bhh0,
           Wih1, Whh1, bih1, bhh1, W_dec, b_dec, _n_tf=S, _n_ar=PRED_LEN - 1):
    from concourse.bass_utils import run_bass_kernel_spmd

    x = np.asarray(x, np.float32)
    weights = _prep_weights(
        np.asarray(W_enc, np.float32), np.asarray(b_enc, np.float32),
        np.asarray(Wih0, np.float32), np.asarray(Whh0, np.float32),
        np.asarray(bih0, np.float32), np.asarray(bhh0, np.float32),
        np.asarray(Wih1, np.float32), np.asarray(Whh1, np.float32),
        np.asarray(bih1, np.float32), np.asarray(bhh1, np.float32),
        np.asarray(W_dec, np.float32), np.asarray(b_dec, np.float32))

    nc = _get_program(_n_tf, _n_ar)
    in_maps = _make_in_maps(x, weights, _n_tf)
    res = run_bass_kernel_spmd(nc, in_maps, core_ids=list(range(NCORES)))

    out = np.empty((B, _n_ar + 1, F), np.float32)
    for c in range(NCORES):
        y = np.asarray(res.results[c]["y"], dtype=np.float32)  # [n_out, F, BL]
        out[c * BL:(c + 1) * BL] = y.transpose(2, 0, 1)
    return out
